# revision 1
# baseline (speedup 1.0000x reference)
"""Cross-attention kernel for Trainium2, sharded over 8 NeuronCores.

Sharding: core c handles batch b = c // 4 and head-group g = c % 4
(4 of 16 heads, i.e. 256 of 1024 channels). Each core computes
  q_g = query[b] @ Wq[g].T ; k_g = key[b] @ Wk[g].T ; v_g = value[b] @ Wv[g].T
  x_g = softmax(q_g k_g^T * scale) v_g          (4 heads, independent)
  partial_g = x_g @ Wp[:, g].T                  (partial over head-group)
Host sums the 4 partials per batch and adds the bias terms
(bp exactly, and bv folded through: softmax rows sum to 1, so the value
bias contributes bv @ Wp.T to every token).

On-chip layout: channel-major ("transposed") activations so every matmul
contracts along SBUF partitions. Scores are computed transposed
(scoresT[m, q]) so the softmax denominator reduces along partitions via a
ones-augmented value matrix (extra column of 1.0 in v), and the PV matmul
chains directly off the exp output. All matmuls run in float32r (full PE
rate at free-dim >= 256).
"""

import numpy as np

import concourse.bass as bass
import concourse.mybir as mybir
import concourse.tile as tile
from concourse import bacc
from concourse.bass_utils import run_bass_kernel_spmd

B, N, DIM, H, DH = 2, 2048, 1024, 16, 64
NCORES = 8
HG = 4            # head-groups (cores per batch)
HPG = H // HG     # heads per group = 4
CS = DIM // HG    # channels per group = 256
P = 128
KT = DIM // P     # 8 contraction tiles for the projections
NT = N // P       # 16 token tiles
QW = 512          # q-chunk width (fp32 moving-operand max)
QC = N // QW      # 4 q-chunks

FP32 = mybir.dt.float32
FP32R = mybir.dt.float32r
AF = mybir.ActivationFunctionType


def _build(scale: float, add_qk_bias: bool, reps: int = 1,
           loop_reps: int | None = None):
    nc = bacc.Bacc("TRN2", target_bir_lowering=False, debug=False,
                   num_devices=NCORES)

    qT = nc.dram_tensor("qT", [DIM, N], FP32R, kind="ExternalInput").ap()
    kT = nc.dram_tensor("kT", [DIM, N], FP32R, kind="ExternalInput").ap()
    vT = nc.dram_tensor("vT", [DIM, N], FP32R, kind="ExternalInput").ap()
    wq = nc.dram_tensor("wq", [DIM, CS], FP32R, kind="ExternalInput").ap()
    wk = nc.dram_tensor("wk", [DIM, CS], FP32R, kind="ExternalInput").ap()
    wv = nc.dram_tensor("wv", [DIM, CS], FP32R, kind="ExternalInput").ap()
    wp = nc.dram_tensor("wp", [CS, DIM], FP32R, kind="ExternalInput").ap()
    bqk = nc.dram_tensor("bqk", [P, 2 * (CS // P)], FP32,
                         kind="ExternalInput").ap()
    out = nc.dram_tensor("out", [DIM, N], FP32, kind="ExternalOutput").ap()

    from contextlib import ExitStack
    with nc.allow_low_precision(reason="fp32r matmul rounding is intended"), \
         tile.TileContext(nc) as tc, ExitStack() as stack:
        wpool = stack.enter_context(tc.tile_pool(name="wpool", bufs=1))
        persist = stack.enter_context(tc.tile_pool(name="persist", bufs=1))
        const = stack.enter_context(tc.tile_pool(name="const", bufs=1))

        # Weights resident in SBUF.
        wq_sb = wpool.tile([P, KT * CS], FP32R, tag="wq")
        wk_sb = wpool.tile([P, KT * CS], FP32R, tag="wk")
        wv_sb = wpool.tile([P, KT * CS], FP32R, tag="wv")
        wp_sb = wpool.tile([P, (CS // P) * DIM], FP32R, tag="wp")
        for k in range(KT):
            nc.sync.dma_start(out=wq_sb[:, k * CS:(k + 1) * CS],
                              in_=wq[k * P:(k + 1) * P, :])
            nc.sync.dma_start(out=wk_sb[:, k * CS:(k + 1) * CS],
                              in_=wk[k * P:(k + 1) * P, :])
            nc.sync.dma_start(out=wv_sb[:, k * CS:(k + 1) * CS],
                              in_=wv[k * P:(k + 1) * P, :])
        for k2 in range(CS // P):
            nc.sync.dma_start(out=wp_sb[:, k2 * DIM:(k2 + 1) * DIM],
                              in_=wp[k2 * P:(k2 + 1) * P, :])
        bqk_sb = const.tile([P, 2 * (CS // P)], FP32, tag="bqk")
        if add_qk_bias:
            nc.sync.dma_start(out=bqk_sb[:], in_=bqk[:])
        ones = const.tile([1, DH], FP32R, tag="ones")
        nc.vector.memset(ones[:].bitcast(FP32), 1.0)

        # Projected activations, channel-major, fp32r.
        qsb = [persist.tile([P, N], FP32R, tag=f"qsb{t}", name=f"qsb{t}") for t in range(2)]
        ksb = [persist.tile([P, N], FP32R, tag=f"ksb{t}", name=f"ksb{t}") for t in range(2)]
        # v token-major with a ones column per head: [tok, 4*(64+1)]
        vsb = [persist.tile([P, HPG * (DH + 1)], FP32R, tag=f"vsb{t}", name=f"vsb{t}")
               for t in range(NT)]

        from contextlib import nullcontext
        loop_cm = (tc.For_i(0, loop_reps, 1) if loop_reps
                   else nullcontext())
        with loop_cm:
          for rep in range(reps):
            # ---- Phase Q / K: channel-major projections -------------------
            def qk_proj(src_dram, w_sb, dst, bias_col):
                with tc.tile_pool(name="stream", bufs=2) as stream, \
                     tc.tile_pool(name="pp", bufs=1, space="PSUM") as pp:
                    pA = pp.tile([P, N], FP32, tag="pA")
                    pB = pp.tile([P, N], FP32, tag="pB")
                    for k in range(KT):
                        ts_ = stream.tile([P, N], FP32R, tag="s")
                        nc.sync.dma_start(out=ts_[:],
                                          in_=src_dram[k * P:(k + 1) * P, :])
                        for nn in range(QC):
                            nc.tensor.matmul(
                                pA[:, nn * QW:(nn + 1) * QW],
                                w_sb[:, k * CS:k * CS + P],
                                ts_[:, nn * QW:(nn + 1) * QW],
                                start=(k == 0), stop=(k == KT - 1))
                            nc.tensor.matmul(
                                pB[:, nn * QW:(nn + 1) * QW],
                                w_sb[:, k * CS + P:(k + 1) * CS],
                                ts_[:, nn * QW:(nn + 1) * QW],
                                start=(k == 0), stop=(k == KT - 1))
                    for t, pt in enumerate((pA, pB)):
                        if add_qk_bias:
                            nc.vector.tensor_scalar(
                                dst[t][:], pt[:],
                                bqk_sb[:, bias_col + t:bias_col + t + 1], None,
                                mybir.AluOpType.add)
                        else:
                            nc.vector.tensor_copy(dst[t][:], pt[:])

            qk_proj(qT, wq_sb, qsb, 0)
            qk_proj(kT, wk_sb, ksb, CS // P)

            # ---- Phase V: token-major projection --------------------------
            # One PSUM bank per token-tile accumulator (start=True clears the
            # whole bank, so accumulation groups must not share one). 8 banks
            # per pass, two passes over a fully resident vT.
            with tc.tile_pool(name="streamv", bufs=1) as stream, \
                 tc.tile_pool(name="pv", bufs=8, space="PSUM") as pvp:
                vres = [stream.tile([P, N], FP32R, tag=f"vres{k}",
                                    name=f"vres{k}_{rep}") for k in range(KT)]
                for k in range(KT):
                    nc.sync.dma_start(out=vres[k][:], in_=vT[k * P:(k + 1) * P, :])
                for half in range(2):
                    pvt = [pvp.tile([P, CS], FP32, tag="pv",
                                    name=f"pv{half}_{t8}_{rep}") for t8 in range(8)]
                    for k in range(KT):
                        for t8 in range(8):
                            tt = half * 8 + t8
                            nc.tensor.matmul(
                                pvt[t8][:],
                                vres[k][:, tt * P:(tt + 1) * P],
                                wv_sb[:, k * CS:(k + 1) * CS],
                                start=(k == 0), stop=(k == KT - 1))
                    for t8 in range(8):
                        tt = half * 8 + t8
                        dst3 = vsb[tt][:].rearrange("p (h c) -> p h c", h=HPG)
                        nc.vector.tensor_copy(
                            dst3[:, :, 0:DH],
                            pvt[t8][:].rearrange("p (h c) -> p h c", h=HPG))
                        nc.vector.memset(dst3[:, :, DH:DH + 1].bitcast(FP32), 1.0)

            # ---- Phase C: attention + output projection, per q-chunk ------
            with tc.tile_pool(name="probs", bufs=3) as probs, \
                 tc.tile_pool(name="xq", bufs=2) as xqp, \
                 tc.tile_pool(name="small", bufs=2) as small, \
                 tc.tile_pool(name="ost", bufs=2) as ostp, \
                 tc.tile_pool(name="psc", bufs=2, space="PSUM") as psc, \
                 tc.tile_pool(name="pxt", bufs=1, space="PSUM") as pxt, \
                 tc.tile_pool(name="pbc", bufs=1, space="PSUM") as pbc, \
                 tc.tile_pool(name="po", bufs=1, space="PSUM") as pop:
                for qq in range(QC):
                    qs = slice(qq * QW, (qq + 1) * QW)
                    xq = [xqp.tile([P, QW], FP32R, tag=f"x{t}", name=f"xq{t}_{qq}_{rep}") for t in range(2)]
                    for hp in range(HPG // 2):
                        # heads A = 2*hp (partitions 0:64 of tile hp),
                        # B = 2*hp+1 (partitions 64:128); their K=64 score
                        # matmuls occupy disjoint PE row-groups and run
                        # concurrently, sharing one [128, 1024] psum tile.
                        pt = hp
                        xtA = pxt.tile([P, QW], FP32, tag="xtA")
                        xtB = pxt.tile([P, QW], FP32, tag="xtB")
                        for m in range(NT):
                            sc = psc.tile([P, 2 * QW], FP32, tag="sc")
                            pr = probs.tile([P, 2 * QW], FP32R, tag="pr")
                            for j, off in ((0, 0), (1, DH)):
                                nc.tensor.matmul(
                                    sc[:, j * QW:(j + 1) * QW],
                                    ksb[pt][off:off + DH, m * P:(m + 1) * P],
                                    qsb[pt][off:off + DH, qs],
                                    start=True, stop=True,
                                    tile_position=(off, 0))
                            nc.scalar.activation(pr[:], sc[:], AF.Exp, scale=scale)
                            for j, xt, h in ((0, xtA, 2 * hp), (1, xtB, 2 * hp + 1)):
                                nc.tensor.matmul(
                                    xt[0:DH + 1, :],
                                    vsb[m][:, h * (DH + 1):(h + 1) * (DH + 1)],
                                    pr[:, j * QW:(j + 1) * QW],
                                    start=(m == 0), stop=(m == NT - 1))
                        for xt, off in ((xtA, 0), (xtB, DH)):
                            # denominator -> SBUF, cheap approx reciprocal
                            den = small.tile([1, QW], FP32, tag="den")
                            nc.vector.tensor_copy(den[:], xt[DH:DH + 1, :])
                            rde = small.tile([1, QW], FP32, tag="rde")
                            nc.vector.reciprocal_approx_fast(out=rde[:], in_=den[:])
                            rdr = small.tile([1, QW], FP32R, tag="rdr")
                            nc.vector.tensor_copy(rdr[:], rde[:])
                            bc = pbc.tile([DH, QW], FP32, tag="bc")
                            nc.tensor.matmul(bc[:], ones[:], rdr[:],
                                             start=True, stop=True)
                            bcs = small.tile([DH, QW], FP32R, tag="bcs")
                            nc.vector.tensor_copy(bcs[:], bc[:])
                            nc.vector.tensor_mul(xq[pt][off:off + DH, :],
                                                 xt[0:DH, :], bcs[:])
                    # output projection for this q-chunk
                    for mo in range(KT):
                        po = pop.tile([P, QW], FP32, tag="po")
                        for k2 in range(CS // P):
                            nc.tensor.matmul(
                                po[:],
                                wp_sb[:, k2 * DIM + mo * P:k2 * DIM + (mo + 1) * P],
                                xq[k2][:],
                                start=(k2 == 0), stop=(k2 == CS // P - 1))
                        ost = ostp.tile([P, QW], FP32, tag="ost")
                        nc.vector.tensor_copy(ost[:], po[:])
                        nc.sync.dma_start(out=out[mo * P:(mo + 1) * P, qs],
                                          in_=ost[:])

    nc.compile()
    return nc


_CACHE = {}


def _get_program(scale: float, add_qk_bias: bool, reps: int = 1,
                 loop_reps=None):
    key = (scale, add_qk_bias, reps, loop_reps)
    if key not in _CACHE:
        _CACHE[key] = _build(scale, add_qk_bias, reps, loop_reps)
    return _CACHE[key]


def make_in_maps(query, key, value, Wq, bq, Wk, bk, Wv, bv, Wp, bp, scale):
    query = np.asarray(query, np.float32)
    key = np.asarray(key, np.float32)
    value = np.asarray(value, np.float32)
    Wq, Wk, Wv, Wp = (np.asarray(a, np.float32) for a in (Wq, Wk, Wv, Wp))
    bq, bk = np.asarray(bq, np.float32), np.asarray(bk, np.float32)
    in_maps = []
    for c in range(NCORES):
        b, g = c // HG, c % HG
        cs = slice(g * CS, (g + 1) * CS)
        bqk_arr = np.stack([bq[cs].reshape(CS // P, P),
                            bk[cs].reshape(CS // P, P)]).reshape(-1, P).T
        in_maps.append({
            "qT": np.ascontiguousarray(query[b].T),
            "kT": np.ascontiguousarray(key[b].T),
            "vT": np.ascontiguousarray(value[b].T),
            "wq": np.ascontiguousarray(Wq[cs, :].T),
            "wk": np.ascontiguousarray(Wk[cs, :].T),
            "wv": np.ascontiguousarray(Wv[cs, :].T),
            "wp": np.ascontiguousarray(Wp[:, cs].T),
            "bqk": np.ascontiguousarray(bqk_arr),
        })
    return in_maps


def combine_outputs(results, bv, bp, Wp):
    bv = np.asarray(bv, np.float32)
    bp = np.asarray(bp, np.float32)
    Wp = np.asarray(Wp, np.float32)
    out = np.empty((B, N, DIM), np.float32)
    corr = bp + bv @ Wp.T
    for b in range(B):
        acc = results[b * HG]["out"].copy()
        for g in range(1, HG):
            acc += results[b * HG + g]["out"]
        out[b] = acc.T + corr
    return out


def kernel(query, key, value, Wq, bq, Wk, bk, Wv, bv, Wp, bp, scale):
    scale_v = float(np.asarray(scale).reshape(-1)[0])
    add_qk_bias = bool(np.any(np.asarray(bq)) or np.any(np.asarray(bk)))
    nc = _get_program(scale_v, add_qk_bias)
    in_maps = make_in_maps(query, key, value, Wq, bq, Wk, bk, Wv, bv,
                           Wp, bp, scale)
    res = run_bass_kernel_spmd(nc, in_maps, list(range(NCORES))).results
    return combine_outputs(res, bv, bp, Wp)



# revision 2
# speedup vs baseline: 57.6483x; 57.6483x over previous
"""Cross-attention kernel for Trainium2, sharded over 8 NeuronCores.

Same structure as the fp32r baseline (core c: batch c//4, head-group
c%4; channel-major activations; transposed scores with ones-augmented
value matrix for the softmax denominator), but all DRAM I/O and SBUF
matmul operands are bfloat16:

  - q/k/v inputs, weights, and the partial outputs move over HBM as
    bf16 — halves the 36 MB/core DMA footprint that was the binding
    roofline (~100 us at ~358 GB/s/core).
  - PE rate is unchanged (fp32r and bf16 both stream 1 column/cycle);
    PSUM accumulation stays fp32.
  - The host sums the four bf16 head-group partials per batch in fp32
    and adds the bias correction (bp + bv @ Wp.T).
"""

import numpy as np
import ml_dtypes

import concourse.bass as bass
import concourse.mybir as mybir
import concourse.tile as tile
from concourse import bacc
from concourse.bass_utils import run_bass_kernel_spmd

B, N, DIM, H, DH = 2, 2048, 1024, 16, 64
NCORES = 8
HG = 4            # head-groups (cores per batch)
HPG = H // HG     # heads per group = 4
CS = DIM // HG    # channels per group = 256
P = 128
KT = DIM // P     # 8 contraction tiles for the projections
NT = N // P       # 16 token tiles
QW = 512          # q-chunk width (one PSUM bank of fp32 output)
QC = N // QW      # 4 q-chunks

FP32 = mybir.dt.float32
BF16 = mybir.dt.bfloat16
AF = mybir.ActivationFunctionType
BFNP = ml_dtypes.bfloat16


def _build(scale: float, add_qk_bias: bool, reps: int = 1,
           loop_reps: int | None = None):
    nc = bacc.Bacc("TRN2", target_bir_lowering=False, debug=False,
                   num_devices=NCORES)

    qT = nc.dram_tensor("qT", [DIM, N], BF16, kind="ExternalInput").ap()
    kT = nc.dram_tensor("kT", [DIM, N], BF16, kind="ExternalInput").ap()
    vT = nc.dram_tensor("vT", [DIM, N], BF16, kind="ExternalInput").ap()
    wq = nc.dram_tensor("wq", [DIM, CS], BF16, kind="ExternalInput").ap()
    wk = nc.dram_tensor("wk", [DIM, CS], BF16, kind="ExternalInput").ap()
    wv = nc.dram_tensor("wv", [DIM, CS], BF16, kind="ExternalInput").ap()
    wp = nc.dram_tensor("wp", [CS, DIM], BF16, kind="ExternalInput").ap()
    bqk = nc.dram_tensor("bqk", [P, 2 * (CS // P)], FP32,
                         kind="ExternalInput").ap()
    out = nc.dram_tensor("out", [DIM, N], BF16, kind="ExternalOutput").ap()

    from contextlib import ExitStack
    with nc.allow_low_precision(reason="bf16 matmul rounding is intended"), \
         tile.TileContext(nc) as tc, ExitStack() as stack:
        wpool = stack.enter_context(tc.tile_pool(name="wpool", bufs=1))
        persist = stack.enter_context(tc.tile_pool(name="persist", bufs=1))
        const = stack.enter_context(tc.tile_pool(name="const", bufs=1))

        # Weights resident in SBUF.
        wq_sb = wpool.tile([P, KT * CS], BF16, tag="wq")
        wk_sb = wpool.tile([P, KT * CS], BF16, tag="wk")
        wv_sb = wpool.tile([P, KT * CS], BF16, tag="wv")
        wp_sb = wpool.tile([P, (CS // P) * DIM], BF16, tag="wp")
        for k in range(KT):
            nc.sync.dma_start(out=wq_sb[:, k * CS:(k + 1) * CS],
                              in_=wq[k * P:(k + 1) * P, :])
            nc.sync.dma_start(out=wk_sb[:, k * CS:(k + 1) * CS],
                              in_=wk[k * P:(k + 1) * P, :])
            nc.sync.dma_start(out=wv_sb[:, k * CS:(k + 1) * CS],
                              in_=wv[k * P:(k + 1) * P, :])
        for k2 in range(CS // P):
            nc.sync.dma_start(out=wp_sb[:, k2 * DIM:(k2 + 1) * DIM],
                              in_=wp[k2 * P:(k2 + 1) * P, :])
        bqk_sb = const.tile([P, 2 * (CS // P)], FP32, tag="bqk")
        if add_qk_bias:
            nc.sync.dma_start(out=bqk_sb[:], in_=bqk[:])
        ones = const.tile([1, DH], BF16, tag="ones")
        nc.vector.memset(ones[:], 1.0)

        # Projected activations, channel-major, bf16.
        qsb = [persist.tile([P, N], BF16, tag=f"qsb{t}", name=f"qsb{t}") for t in range(2)]
        ksb = [persist.tile([P, N], BF16, tag=f"ksb{t}", name=f"ksb{t}") for t in range(2)]
        # v token-major with a ones column per head: [tok, 4*(64+1)]
        vsb = [persist.tile([P, HPG * (DH + 1)], BF16, tag=f"vsb{t}", name=f"vsb{t}")
               for t in range(NT)]

        from contextlib import nullcontext
        loop_cm = (tc.For_i(0, loop_reps, 1) if loop_reps
                   else nullcontext())
        with loop_cm:
          for rep in range(reps):
            # ---- Phase Q / K: channel-major projections -------------------
            def qk_proj(src_dram, w_sb, dst, bias_col):
                with tc.tile_pool(name="stream", bufs=2) as stream, \
                     tc.tile_pool(name="pp", bufs=1, space="PSUM") as pp:
                    pA = pp.tile([P, N], FP32, tag="pA")
                    pB = pp.tile([P, N], FP32, tag="pB")
                    for k in range(KT):
                        ts_ = stream.tile([P, N], BF16, tag="s")
                        nc.sync.dma_start(out=ts_[:],
                                          in_=src_dram[k * P:(k + 1) * P, :])
                        for nn in range(QC):
                            nc.tensor.matmul(
                                pA[:, nn * QW:(nn + 1) * QW],
                                w_sb[:, k * CS:k * CS + P],
                                ts_[:, nn * QW:(nn + 1) * QW],
                                start=(k == 0), stop=(k == KT - 1))
                            nc.tensor.matmul(
                                pB[:, nn * QW:(nn + 1) * QW],
                                w_sb[:, k * CS + P:(k + 1) * CS],
                                ts_[:, nn * QW:(nn + 1) * QW],
                                start=(k == 0), stop=(k == KT - 1))
                    for t, pt in enumerate((pA, pB)):
                        if add_qk_bias:
                            nc.vector.tensor_scalar(
                                dst[t][:], pt[:],
                                bqk_sb[:, bias_col + t:bias_col + t + 1], None,
                                mybir.AluOpType.add)
                        else:
                            nc.vector.tensor_copy(dst[t][:], pt[:])

            qk_proj(qT, wq_sb, qsb, 0)
            qk_proj(kT, wk_sb, ksb, CS // P)

            # ---- Phase V: token-major projection --------------------------
            # One PSUM bank per token-tile accumulator (start=True clears the
            # whole bank, so accumulation groups must not share one). 8 banks
            # per pass, two passes over a fully resident vT.
            with tc.tile_pool(name="streamv", bufs=1) as stream, \
                 tc.tile_pool(name="pv", bufs=8, space="PSUM") as pvp:
                vres = [stream.tile([P, N], BF16, tag=f"vres{k}",
                                    name=f"vres{k}_{rep}") for k in range(KT)]
                for k in range(KT):
                    nc.sync.dma_start(out=vres[k][:], in_=vT[k * P:(k + 1) * P, :])
                for half in range(2):
                    pvt = [pvp.tile([P, CS], FP32, tag="pv",
                                    name=f"pv{half}_{t8}_{rep}") for t8 in range(8)]
                    for k in range(KT):
                        for t8 in range(8):
                            tt = half * 8 + t8
                            nc.tensor.matmul(
                                pvt[t8][:],
                                vres[k][:, tt * P:(tt + 1) * P],
                                wv_sb[:, k * CS:(k + 1) * CS],
                                start=(k == 0), stop=(k == KT - 1))
                    for t8 in range(8):
                        tt = half * 8 + t8
                        dst3 = vsb[tt][:].rearrange("p (h c) -> p h c", h=HPG)
                        nc.vector.tensor_copy(
                            dst3[:, :, 0:DH],
                            pvt[t8][:].rearrange("p (h c) -> p h c", h=HPG))
                        nc.vector.memset(dst3[:, :, DH:DH + 1], 1.0)

            # ---- Phase C: attention + output projection, per q-chunk ------
            with tc.tile_pool(name="probs", bufs=3) as probs, \
                 tc.tile_pool(name="xq", bufs=2) as xqp, \
                 tc.tile_pool(name="small", bufs=2) as small, \
                 tc.tile_pool(name="ost", bufs=2) as ostp, \
                 tc.tile_pool(name="psc", bufs=2, space="PSUM") as psc, \
                 tc.tile_pool(name="pxt", bufs=1, space="PSUM") as pxt, \
                 tc.tile_pool(name="pbc", bufs=1, space="PSUM") as pbc, \
                 tc.tile_pool(name="po", bufs=1, space="PSUM") as pop:
                for qq in range(QC):
                    qs = slice(qq * QW, (qq + 1) * QW)
                    xq = [xqp.tile([P, QW], BF16, tag=f"x{t}", name=f"xq{t}_{qq}_{rep}") for t in range(2)]
                    for hp in range(HPG // 2):
                        # heads A = 2*hp (partitions 0:64 of tile hp),
                        # B = 2*hp+1 (partitions 64:128); their K=64 score
                        # matmuls occupy disjoint PE row-groups and run
                        # concurrently, sharing one [128, 1024] psum tile.
                        pt = hp
                        xtA = pxt.tile([P, QW], FP32, tag="xtA")
                        xtB = pxt.tile([P, QW], FP32, tag="xtB")
                        for m in range(NT):
                            sc = psc.tile([P, 2 * QW], FP32, tag="sc")
                            pr = probs.tile([P, 2 * QW], BF16, tag="pr")
                            for j, off in ((0, 0), (1, DH)):
                                nc.tensor.matmul(
                                    sc[:, j * QW:(j + 1) * QW],
                                    ksb[pt][off:off + DH, m * P:(m + 1) * P],
                                    qsb[pt][off:off + DH, qs],
                                    start=True, stop=True,
                                    tile_position=(off, 0))
                            nc.scalar.activation(pr[:], sc[:], AF.Exp, scale=scale)
                            for j, xt, h in ((0, xtA, 2 * hp), (1, xtB, 2 * hp + 1)):
                                nc.tensor.matmul(
                                    xt[0:DH + 1, :],
                                    vsb[m][:, h * (DH + 1):(h + 1) * (DH + 1)],
                                    pr[:, j * QW:(j + 1) * QW],
                                    start=(m == 0), stop=(m == NT - 1))
                        for xt, off in ((xtA, 0), (xtB, DH)):
                            # denominator -> SBUF, cheap approx reciprocal
                            den = small.tile([1, QW], FP32, tag="den")
                            nc.vector.tensor_copy(den[:], xt[DH:DH + 1, :])
                            rde = small.tile([1, QW], FP32, tag="rde")
                            nc.vector.reciprocal_approx_fast(out=rde[:], in_=den[:])
                            rdr = small.tile([1, QW], BF16, tag="rdr")
                            nc.vector.tensor_copy(rdr[:], rde[:])
                            bc = pbc.tile([DH, QW], FP32, tag="bc")
                            nc.tensor.matmul(bc[:], ones[:], rdr[:],
                                             start=True, stop=True)
                            bcs = small.tile([DH, QW], BF16, tag="bcs")
                            nc.vector.tensor_copy(bcs[:], bc[:])
                            nc.vector.tensor_mul(xq[pt][off:off + DH, :],
                                                 xt[0:DH, :], bcs[:])
                    # output projection for this q-chunk
                    for mo in range(KT):
                        po = pop.tile([P, QW], FP32, tag="po")
                        for k2 in range(CS // P):
                            nc.tensor.matmul(
                                po[:],
                                wp_sb[:, k2 * DIM + mo * P:k2 * DIM + (mo + 1) * P],
                                xq[k2][:],
                                start=(k2 == 0), stop=(k2 == CS // P - 1))
                        ost = ostp.tile([P, QW], BF16, tag="ost")
                        nc.vector.tensor_copy(ost[:], po[:])
                        nc.sync.dma_start(out=out[mo * P:(mo + 1) * P, qs],
                                          in_=ost[:])

    nc.compile()
    return nc


_CACHE = {}


def _get_program(scale: float, add_qk_bias: bool, reps: int = 1,
                 loop_reps=None):
    key = (scale, add_qk_bias, reps, loop_reps)
    if key not in _CACHE:
        _CACHE[key] = _build(scale, add_qk_bias, reps, loop_reps)
    return _CACHE[key]


def make_in_maps(query, key, value, Wq, bq, Wk, bk, Wv, bv, Wp, bp, scale):
    query = np.asarray(query, np.float32)
    key = np.asarray(key, np.float32)
    value = np.asarray(value, np.float32)
    Wq, Wk, Wv, Wp = (np.asarray(a, np.float32) for a in (Wq, Wk, Wv, Wp))
    bq, bk = np.asarray(bq, np.float32), np.asarray(bk, np.float32)
    qTb = [np.ascontiguousarray(query[b].T).astype(BFNP) for b in range(B)]
    kTb = [np.ascontiguousarray(key[b].T).astype(BFNP) for b in range(B)]
    vTb = [np.ascontiguousarray(value[b].T).astype(BFNP) for b in range(B)]
    in_maps = []
    for c in range(NCORES):
        b, g = c // HG, c % HG
        cs = slice(g * CS, (g + 1) * CS)
        bqk_arr = np.stack([bq[cs].reshape(CS // P, P),
                            bk[cs].reshape(CS // P, P)]).reshape(-1, P).T
        in_maps.append({
            "qT": qTb[b],
            "kT": kTb[b],
            "vT": vTb[b],
            "wq": np.ascontiguousarray(Wq[cs, :].T).astype(BFNP),
            "wk": np.ascontiguousarray(Wk[cs, :].T).astype(BFNP),
            "wv": np.ascontiguousarray(Wv[cs, :].T).astype(BFNP),
            "wp": np.ascontiguousarray(Wp[:, cs].T).astype(BFNP),
            "bqk": np.ascontiguousarray(bqk_arr),
        })
    return in_maps


def combine_outputs(results, bv, bp, Wp):
    bv = np.asarray(bv, np.float32)
    bp = np.asarray(bp, np.float32)
    Wp = np.asarray(Wp, np.float32)
    out = np.empty((B, N, DIM), np.float32)
    corr = bp + bv @ Wp.T
    for b in range(B):
        acc = results[b * HG]["out"].astype(np.float32)
        for g in range(1, HG):
            acc += results[b * HG + g]["out"].astype(np.float32)
        out[b] = acc.T + corr
    return out


def kernel(query, key, value, Wq, bq, Wk, bk, Wv, bv, Wp, bp, scale):
    scale_v = float(np.asarray(scale).reshape(-1)[0])
    add_qk_bias = bool(np.any(np.asarray(bq)) or np.any(np.asarray(bk)))
    nc = _get_program(scale_v, add_qk_bias)
    in_maps = make_in_maps(query, key, value, Wq, bq, Wk, bk, Wv, bv,
                           Wp, bp, scale)
    res = run_bass_kernel_spmd(nc, in_maps, list(range(NCORES))).results
    return combine_outputs(res, bv, bp, Wp)


# revision 4
# speedup vs baseline: 60.1584x; 1.0435x over previous
"""Cross-attention kernel for Trainium2, sharded over 8 NeuronCores.

Same structure as the fp32r baseline (core c: batch c//4, head-group
c%4; channel-major activations; transposed scores with ones-augmented
value matrix for the softmax denominator), but all DRAM I/O and SBUF
matmul operands are bfloat16:

  - q/k/v inputs, weights, and the partial outputs move over HBM as
    bf16 — halves the 36 MB/core DMA footprint that was the binding
    roofline (~100 us at ~358 GB/s/core).
  - PE rate is unchanged (fp32r and bf16 both stream 1 column/cycle);
    PSUM accumulation stays fp32.
  - The host sums the four bf16 head-group partials per batch in fp32
    and adds the bias correction (bp + bv @ Wp.T).
"""

import numpy as np
import ml_dtypes

import concourse.bass as bass
import concourse.mybir as mybir
import concourse.tile as tile
from concourse import bacc
from concourse.bass_utils import run_bass_kernel_spmd

B, N, DIM, H, DH = 2, 2048, 1024, 16, 64
NCORES = 8
HG = 4            # head-groups (cores per batch)
HPG = H // HG     # heads per group = 4
CS = DIM // HG    # channels per group = 256
P = 128
KT = DIM // P     # 8 contraction tiles for the projections
NT = N // P       # 16 token tiles
QW = 512          # q-chunk width (one PSUM bank of fp32 output)
QC = N // QW      # 4 q-chunks

FP32 = mybir.dt.float32
BF16 = mybir.dt.bfloat16
AF = mybir.ActivationFunctionType
BFNP = ml_dtypes.bfloat16


def _build(scale: float, add_qk_bias: bool, reps: int = 1,
           loop_reps: int | None = None):
    nc = bacc.Bacc("TRN2", target_bir_lowering=False, debug=False,
                   num_devices=NCORES)

    qT = nc.dram_tensor("qT", [DIM, N], BF16, kind="ExternalInput").ap()
    kT = nc.dram_tensor("kT", [DIM, N], BF16, kind="ExternalInput").ap()
    vT = nc.dram_tensor("vT", [DIM, N], BF16, kind="ExternalInput").ap()
    wq = nc.dram_tensor("wq", [DIM, CS], BF16, kind="ExternalInput").ap()
    wk = nc.dram_tensor("wk", [DIM, CS], BF16, kind="ExternalInput").ap()
    wv = nc.dram_tensor("wv", [DIM, CS], BF16, kind="ExternalInput").ap()
    wp = nc.dram_tensor("wp", [CS, DIM], BF16, kind="ExternalInput").ap()
    bqk = nc.dram_tensor("bqk", [P, 2 * (CS // P)], FP32,
                         kind="ExternalInput").ap()
    out = nc.dram_tensor("out", [DIM, N], BF16, kind="ExternalOutput").ap()

    from contextlib import ExitStack
    with nc.allow_low_precision(reason="bf16 matmul rounding is intended"), \
         tile.TileContext(nc) as tc, ExitStack() as stack:
        wpool = stack.enter_context(tc.tile_pool(name="wpool", bufs=1))
        persist = stack.enter_context(tc.tile_pool(name="persist", bufs=1))
        const = stack.enter_context(tc.tile_pool(name="const", bufs=1))

        # Weights resident in SBUF.
        wq_sb = wpool.tile([P, KT * CS], BF16, tag="wq")
        wk_sb = wpool.tile([P, KT * CS], BF16, tag="wk")
        wv_sb = wpool.tile([P, KT * CS], BF16, tag="wv")
        wp_sb = wpool.tile([P, (CS // P) * DIM], BF16, tag="wp")
        for k in range(KT):
            nc.sync.dma_start(out=wq_sb[:, k * CS:(k + 1) * CS],
                              in_=wq[k * P:(k + 1) * P, :])
            nc.sync.dma_start(out=wk_sb[:, k * CS:(k + 1) * CS],
                              in_=wk[k * P:(k + 1) * P, :])
            nc.sync.dma_start(out=wv_sb[:, k * CS:(k + 1) * CS],
                              in_=wv[k * P:(k + 1) * P, :])
        for k2 in range(CS // P):
            nc.sync.dma_start(out=wp_sb[:, k2 * DIM:(k2 + 1) * DIM],
                              in_=wp[k2 * P:(k2 + 1) * P, :])
        bqk_sb = const.tile([P, 2 * (CS // P)], FP32, tag="bqk")
        if add_qk_bias:
            nc.sync.dma_start(out=bqk_sb[:], in_=bqk[:])
        ones = const.tile([1, DH], BF16, tag="ones")
        nc.vector.memset(ones[:], 1.0)

        # Projected activations, channel-major, bf16.
        qsb = [persist.tile([P, N], BF16, tag=f"qsb{t}", name=f"qsb{t}") for t in range(2)]
        ksb = [persist.tile([P, N], BF16, tag=f"ksb{t}", name=f"ksb{t}") for t in range(2)]
        # v token-major with a ones column per head: [tok, 4*(64+1)]
        vsb = [persist.tile([P, HPG * (DH + 1)], BF16, tag=f"vsb{t}", name=f"vsb{t}")
               for t in range(NT)]

        from contextlib import nullcontext
        loop_cm = (tc.For_i(0, loop_reps, 1) if loop_reps
                   else nullcontext())
        with loop_cm:
          for rep in range(reps):
            # ---- Phase Q / K: channel-major projections -------------------
            def qk_proj(src_dram, w_sb, dst, bias_col):
                with tc.tile_pool(name="stream", bufs=2) as stream, \
                     tc.tile_pool(name="pp", bufs=1, space="PSUM") as pp:
                    pA = pp.tile([P, N], FP32, tag="pA")
                    pB = pp.tile([P, N], FP32, tag="pB")
                    for k in range(KT):
                        ts_ = stream.tile([P, N], BF16, tag="s")
                        nc.sync.dma_start(out=ts_[:],
                                          in_=src_dram[k * P:(k + 1) * P, :])
                        for nn in range(QC):
                            nc.tensor.matmul(
                                pA[:, nn * QW:(nn + 1) * QW],
                                w_sb[:, k * CS:k * CS + P],
                                ts_[:, nn * QW:(nn + 1) * QW],
                                start=(k == 0), stop=(k == KT - 1))
                            nc.tensor.matmul(
                                pB[:, nn * QW:(nn + 1) * QW],
                                w_sb[:, k * CS + P:(k + 1) * CS],
                                ts_[:, nn * QW:(nn + 1) * QW],
                                start=(k == 0), stop=(k == KT - 1))
                    for t, pt in enumerate((pA, pB)):
                        if add_qk_bias:
                            nc.vector.tensor_scalar(
                                dst[t][:], pt[:],
                                bqk_sb[:, bias_col + t:bias_col + t + 1], None,
                                mybir.AluOpType.add)
                        else:
                            nc.vector.tensor_copy(dst[t][:], pt[:])

            qk_proj(qT, wq_sb, qsb, 0)
            qk_proj(kT, wk_sb, ksb, CS // P)

            # ---- Phase V: token-major projection --------------------------
            # One PSUM bank per token-tile accumulator (start=True clears the
            # whole bank, so accumulation groups must not share one). 8 banks
            # per pass, two passes over a fully resident vT.
            with tc.tile_pool(name="streamv", bufs=1) as stream, \
                 tc.tile_pool(name="pv", bufs=8, space="PSUM") as pvp:
                vres = [stream.tile([P, N], BF16, tag=f"vres{k}",
                                    name=f"vres{k}_{rep}") for k in range(KT)]
                for k in range(KT):
                    nc.sync.dma_start(out=vres[k][:], in_=vT[k * P:(k + 1) * P, :])
                for half in range(2):
                    pvt = [pvp.tile([P, CS], FP32, tag="pv",
                                    name=f"pv{half}_{t8}_{rep}") for t8 in range(8)]
                    for k in range(KT):
                        for t8 in range(8):
                            tt = half * 8 + t8
                            nc.tensor.matmul(
                                pvt[t8][:],
                                vres[k][:, tt * P:(tt + 1) * P],
                                wv_sb[:, k * CS:(k + 1) * CS],
                                start=(k == 0), stop=(k == KT - 1))
                    for t8 in range(8):
                        tt = half * 8 + t8
                        dst3 = vsb[tt][:].rearrange("p (h c) -> p h c", h=HPG)
                        nc.vector.tensor_copy(
                            dst3[:, :, 0:DH],
                            pvt[t8][:].rearrange("p (h c) -> p h c", h=HPG))
                        nc.vector.memset(dst3[:, :, DH:DH + 1], 1.0)

            # ---- Phase C: attention + output projection, per q-chunk ------
            with tc.tile_pool(name="probs", bufs=3) as probs, \
                 tc.tile_pool(name="xq", bufs=2) as xqp, \
                 tc.tile_pool(name="small", bufs=2) as small, \
                 tc.tile_pool(name="ost", bufs=2) as ostp, \
                 tc.tile_pool(name="psc", bufs=2, space="PSUM") as psc, \
                 tc.tile_pool(name="pxt", bufs=1, space="PSUM") as pxt, \
                 tc.tile_pool(name="pbc", bufs=1, space="PSUM") as pbc, \
                 tc.tile_pool(name="po", bufs=1, space="PSUM") as pop:
                for qq in range(QC):
                    qs = slice(qq * QW, (qq + 1) * QW)
                    xq = [xqp.tile([P, QW], BF16, tag=f"x{t}", name=f"xq{t}_{qq}_{rep}") for t in range(2)]
                    for hp in range(HPG // 2):
                        # heads A = 2*hp (partitions 0:64 of tile hp),
                        # B = 2*hp+1 (partitions 64:128); their K=64 score
                        # matmuls occupy disjoint PE row-groups and run
                        # concurrently, sharing one [128, 1024] psum tile.
                        pt = hp
                        xtA = pxt.tile([P, QW], FP32, tag="xtA")
                        xtB = pxt.tile([P, QW], FP32, tag="xtB")
                        for m in range(NT):
                            sc = psc.tile([P, 2 * QW], FP32, tag="sc")
                            pr = probs.tile([P, 2 * QW], BF16, tag="pr")
                            for j, off in ((0, 0), (1, DH)):
                                nc.tensor.matmul(
                                    sc[:, j * QW:(j + 1) * QW],
                                    ksb[pt][off:off + DH, m * P:(m + 1) * P],
                                    qsb[pt][off:off + DH, qs],
                                    start=True, stop=True,
                                    tile_position=(off, 0))
                            nc.scalar.activation(pr[:], sc[:], AF.Exp, scale=scale)
                            for j, xt, h in ((0, xtA, 2 * hp), (1, xtB, 2 * hp + 1)):
                                nc.tensor.matmul(
                                    xt[0:DH + 1, :],
                                    vsb[m][:, h * (DH + 1):(h + 1) * (DH + 1)],
                                    pr[:, j * QW:(j + 1) * QW],
                                    start=(m == 0), stop=(m == NT - 1))
                        for xt, off in ((xtA, 0), (xtB, DH)):
                            # denominator -> SBUF, cheap approx reciprocal
                            den = small.tile([1, QW], FP32, tag="den")
                            nc.vector.tensor_copy(den[:], xt[DH:DH + 1, :])
                            rde = small.tile([1, QW], FP32, tag="rde")
                            nc.vector.reciprocal_approx_fast(out=rde[:], in_=den[:])
                            rdr = small.tile([1, QW], BF16, tag="rdr")
                            nc.vector.tensor_copy(rdr[:], rde[:])
                            bc = pbc.tile([DH, QW], FP32, tag="bc")
                            nc.tensor.matmul(bc[:], ones[:], rdr[:],
                                             start=True, stop=True)
                            bcs = small.tile([DH, QW], BF16, tag="bcs")
                            nc.vector.tensor_copy(bcs[:], bc[:])
                            nc.vector.tensor_mul(xq[pt][off:off + DH, :],
                                                 xt[0:DH, :], bcs[:])
                    # output projection for this q-chunk
                    for mo in range(KT):
                        po = pop.tile([P, QW], FP32, tag="po")
                        for k2 in range(CS // P):
                            nc.tensor.matmul(
                                po[:],
                                wp_sb[:, k2 * DIM + mo * P:k2 * DIM + (mo + 1) * P],
                                xq[k2][:],
                                start=(k2 == 0), stop=(k2 == CS // P - 1))
                        ost = ostp.tile([P, QW], BF16, tag="ost")
                        nc.vector.tensor_copy(ost[:], po[:])
                        nc.sync.dma_start(out=out[mo * P:(mo + 1) * P, qs],
                                          in_=ost[:])

    nc.compile()
    return nc


_CACHE = {}


def _get_program(scale: float, add_qk_bias: bool, reps: int = 1,
                 loop_reps=None):
    key = (scale, add_qk_bias, reps, loop_reps)
    if key not in _CACHE:
        _CACHE[key] = _build(scale, add_qk_bias, reps, loop_reps)
    return _CACHE[key]


def make_in_maps(query, key, value, Wq, bq, Wk, bk, Wv, bv, Wp, bp, scale):
    query = np.asarray(query, np.float32)
    key = np.asarray(key, np.float32)
    value = np.asarray(value, np.float32)
    Wq, Wk, Wv, Wp = (np.asarray(a, np.float32) for a in (Wq, Wk, Wv, Wp))
    bq, bk = np.asarray(bq, np.float32), np.asarray(bk, np.float32)
    qTb = [np.ascontiguousarray(query[b].T).astype(BFNP) for b in range(B)]
    kTb = [np.ascontiguousarray(key[b].T).astype(BFNP) for b in range(B)]
    vTb = [np.ascontiguousarray(value[b].T).astype(BFNP) for b in range(B)]
    in_maps = []
    for c in range(NCORES):
        b, g = c // HG, c % HG
        cs = slice(g * CS, (g + 1) * CS)
        bqk_arr = np.stack([bq[cs].reshape(CS // P, P),
                            bk[cs].reshape(CS // P, P)]).reshape(-1, P).T
        in_maps.append({
            "qT": qTb[b],
            "kT": kTb[b],
            "vT": vTb[b],
            "wq": np.ascontiguousarray(Wq[cs, :].T).astype(BFNP),
            "wk": np.ascontiguousarray(Wk[cs, :].T).astype(BFNP),
            "wv": np.ascontiguousarray(Wv[cs, :].T).astype(BFNP),
            "wp": np.ascontiguousarray(Wp[:, cs].T).astype(BFNP),
            "bqk": np.ascontiguousarray(bqk_arr),
        })
    return in_maps


def combine_outputs(results, bv, bp, Wp):
    bv = np.asarray(bv, np.float32)
    bp = np.asarray(bp, np.float32)
    Wp = np.asarray(Wp, np.float32)
    out = np.empty((B, N, DIM), np.float32)
    corr = bp + bv @ Wp.T
    for b in range(B):
        acc = results[b * HG]["out"].astype(np.float32)
        for g in range(1, HG):
            acc += results[b * HG + g]["out"].astype(np.float32)
        out[b] = acc.T + corr
    return out


def kernel(query, key, value, Wq, bq, Wk, bk, Wv, bv, Wp, bp, scale):
    scale_v = float(np.asarray(scale).reshape(-1)[0])
    add_qk_bias = bool(np.any(np.asarray(bq)) or np.any(np.asarray(bk)))
    nc = _get_program(scale_v, add_qk_bias)
    in_maps = make_in_maps(query, key, value, Wq, bq, Wk, bk, Wv, bv,
                           Wp, bp, scale)
    res = run_bass_kernel_spmd(nc, in_maps, list(range(NCORES))).results
    return combine_outputs(res, bv, bp, Wp)
